# revision 1
# baseline (speedup 1.0000x reference)
"""Trainium2 Bass kernel: embedding gather + 2-layer MLP (relu), data-parallel on 8 cores.

Reference computation:
    x   = entity_embedding[idx0, idx1, :]        # [B, 128]  gather
    h   = relu(x @ w1.T + b1)                    # [B, 256]
    out = relu(h @ w2.T + b2)                    # [B, 86]

Shapes (hardcoded): entity_embedding [500000, 4, 128] f32, B = 131072.

Strategy:
  - Flatten the table to [2e6, 128]; flat row index = idx0*4 + idx1 (fits int32).
  - Shard the batch 8 ways (16384 rows/core); replicate table + weights.
  - Per core: gather rows via gpsimd indirect DMA into [128, j, 128] SBUF tiles
    (batch on partitions), transpose 128x128 sub-tiles on TensorE so features
    land on partitions, then run the MLP with batch on the free dim:
        hT[256h, b]  = w1 @ xT  (2 matmuls, N=512 free)
        outT[86, b]  = w2 @ hT  (2 accumulating matmuls)
    Biases are per-partition vectors in this orientation; relu+bias fuse into
    one ACT/DVE op per tile. Output is written transposed [86, 16384] per core
    and un-transposed on the host during unshard.
"""

import numpy as np
from contextlib import ExitStack

import concourse.bass as bass
import concourse.bacc as bacc
import concourse.tile as tile
from concourse import mybir
from concourse.bass_utils import run_bass_kernel_spmd
from concourse.masks import make_identity

F32 = mybir.dt.float32
I32 = mybir.dt.int32

N_CORES = 8
B = 131072
BC = B // N_CORES          # 16384 batch rows per core
FEAT = 128
NHID = 256
NOUT = 86
NROWS = 500000 * 4         # flattened table rows
P = 128
NJ = BC // P               # 128 j-columns of gathered rows per core
NJG = 16                   # j-columns per indirect-DMA gather call (2048 rows)
CHUNK_J = 4                # j-columns per MLP chunk (512 batch)
NCHUNK = NJ // CHUNK_J     # 32 chunks per core


def _build_program():
    nc = bacc.Bacc("TRN2", num_devices=N_CORES, num_swdge_queues=4)

    table = nc.dram_tensor("table", [NROWS, FEAT], F32, kind="ExternalInput").ap()
    idxs = nc.dram_tensor("idxs", [P, NJ], I32, kind="ExternalInput").ap()
    w1t = nc.dram_tensor("w1t", [FEAT, NHID], F32, kind="ExternalInput").ap()
    w2t = nc.dram_tensor("w2t", [NHID, NOUT], F32, kind="ExternalInput").ap()
    b1v = nc.dram_tensor("b1v", [NHID], F32, kind="ExternalInput").ap()
    b2v = nc.dram_tensor("b2v", [NOUT], F32, kind="ExternalInput").ap()
    outT = nc.dram_tensor("outT", [NOUT, BC], F32, kind="ExternalOutput").ap()

    with tile.TileContext(nc) as tc, ExitStack() as ctx:
        const = ctx.enter_context(tc.tile_pool(name="const", bufs=1))
        gpool = ctx.enter_context(tc.tile_pool(name="gather", bufs=4))
        xpool = ctx.enter_context(tc.tile_pool(name="xt", bufs=3))
        hpool = ctx.enter_context(tc.tile_pool(name="ht", bufs=3))
        opool = ctx.enter_context(tc.tile_pool(name="ot", bufs=3))
        psum = ctx.enter_context(tc.tile_pool(name="psum", bufs=2, space="PSUM"))

        idt = const.tile([P, P], F32)
        make_identity(nc, idt[:])

        w1t_t = const.tile([FEAT, NHID], F32)
        nc.sync.dma_start(w1t_t[:], w1t[:])
        w2t_t = const.tile([P, NHID // P, NOUT], F32)
        nc.sync.dma_start(w2t_t[:], w2t.rearrange("(k p) n -> p k n", p=P))
        b1_t = const.tile([P, NHID // P], F32)
        nc.sync.dma_start(b1_t[:], b1v.rearrange("(k p) -> p k", p=P))
        b2_t = const.tile([NOUT, 1], F32)
        nc.sync.dma_start(b2_t[:], b2v.rearrange("(n one) -> n one", one=1))
        idx_t = const.tile([P, NJ], I32)
        nc.sync.dma_start(idx_t[:], idxs[:])

        for c in range(NCHUNK):
            bcol = c * CHUNK_J * P  # column base in outT for this chunk
            # HW indirect DMA consumes ONE index per partition; gather the
            # chunk's 512 rows as CHUNK_J calls of 128 rows each.
            gt = gpool.tile([P, CHUNK_J, FEAT], F32)
            for i in range(CHUNK_J):
                j = c * CHUNK_J + i
                ginst = nc.gpsimd.indirect_dma_start(
                    out=gt[:, i, :],
                    out_offset=None,
                    in_=table[:],
                    in_offset=bass.IndirectOffsetOnAxis(
                        ap=idx_t[:, j:j + 1], axis=0
                    ),
                )
                # Spread descriptor generation over the 4 SWDGE queues
                # (parallel Q7 pairs + separate DMA rings).
                ginst.ins.queue = f"qPoolDynamic{i or ''}"
            if True:
                # Transpose 4x [128b, 128f] -> [128f, 128b] into one PSUM bank
                xtr = psum.tile([P, CHUNK_J * P], F32, tag="xtr")
                for i in range(CHUNK_J):
                    nc.tensor.transpose(
                        out=xtr[:, i * P:(i + 1) * P],
                        in_=gt[:, i, :],
                        identity=idt[:],
                    )
                xt = xpool.tile([P, CHUNK_J * P], F32)
                nc.vector.tensor_copy(out=xt[:], in_=xtr[:])

                # lin1: hT[k] = relu(w1[k] @ xT + b1[k]), k over 2 hid halves
                ht = hpool.tile([P, NHID // P, CHUNK_J * P], F32)
                for k in range(NHID // P):
                    hp = psum.tile([P, CHUNK_J * P], F32, tag=f"h{k}")
                    nc.tensor.matmul(
                        out=hp[:],
                        lhsT=w1t_t[:, k * P:(k + 1) * P],
                        rhs=xt[:],
                        start=True,
                        stop=True,
                    )
                    if k == 0:
                        nc.scalar.activation(
                            out=ht[:, k, :], in_=hp[:],
                            func=mybir.ActivationFunctionType.Relu,
                            bias=b1_t[:, k:k + 1],
                        )
                    else:
                        nc.vector.tensor_scalar(
                            out=ht[:, k, :], in0=hp[:],
                            scalar1=b1_t[:, k:k + 1], scalar2=0.0,
                            op0=mybir.AluOpType.add, op1=mybir.AluOpType.max,
                        )

                # lin2: outT = relu(w2 @ hT + b2), accumulate over 2 k-tiles
                op_ = psum.tile([NOUT, CHUNK_J * P], F32, tag="ot")
                for k in range(NHID // P):
                    nc.tensor.matmul(
                        out=op_[:],
                        lhsT=w2t_t[:, k, :],
                        rhs=ht[:, k, :],
                        start=(k == 0),
                        stop=(k == NHID // P - 1),
                    )
                ot = opool.tile([NOUT, CHUNK_J * P], F32)
                nc.scalar.activation(
                    out=ot[:], in_=op_[:],
                    func=mybir.ActivationFunctionType.Relu,
                    bias=b2_t[:],
                )
                nc.sync.dma_start(outT[:, bcol:bcol + CHUNK_J * P], ot[:])

    nc.compile()
    return nc


TRACE = False          # set by test harness to capture an NTFF profile
RUN_KWARGS = None      # extra kwargs for run_bass_kernel_spmd (test harness)
LAST = None            # last BassKernelResults (test harness reads exec_time_ns)

_SLOT_TO_BATCH = None


def _slot_map():
    """local batch index for gather slot (p, j): out column c*512 + i*128 + p
    where j = c*CHUNK_J + i must equal the local batch index."""
    global _SLOT_TO_BATCH
    if _SLOT_TO_BATCH is None:
        pp, jj = np.meshgrid(np.arange(P), np.arange(NJ), indexing="ij")
        cc = jj // CHUNK_J
        ii = jj % CHUNK_J
        _SLOT_TO_BATCH = cc * (CHUNK_J * P) + ii * P + pp  # [P, NJ]
    return _SLOT_TO_BATCH


def kernel(entity_embedding, w1, b1, w2, b2, idx0, idx1):
    table = np.ascontiguousarray(
        np.asarray(entity_embedding, dtype=np.float32).reshape(NROWS, FEAT)
    )
    flat_idx = (np.asarray(idx0, dtype=np.int64) * 4
                + np.asarray(idx1, dtype=np.int64)).astype(np.int32)
    w1t = np.ascontiguousarray(np.asarray(w1, dtype=np.float32).T)
    w2t = np.ascontiguousarray(np.asarray(w2, dtype=np.float32).T)
    b1v = np.ascontiguousarray(np.asarray(b1, dtype=np.float32))
    b2v = np.ascontiguousarray(np.asarray(b2, dtype=np.float32))

    slot = _slot_map()
    in_maps = []
    for core in range(N_CORES):
        local = flat_idx[core * BC:(core + 1) * BC]
        idxs = np.ascontiguousarray(local[slot])  # [P, NJ] int32
        in_maps.append({
            "table": table,
            "idxs": idxs,
            "w1t": w1t,
            "w2t": w2t,
            "b1v": b1v,
            "b2v": b2v,
        })

    nc = _build_program()
    global LAST
    res = run_bass_kernel_spmd(
        nc, in_maps, core_ids=list(range(N_CORES)), trace=TRACE,
        **(RUN_KWARGS or {}),
    )
    LAST = res
    out = np.empty((B, NOUT), dtype=np.float32)
    for core in range(N_CORES):
        out[core * BC:(core + 1) * BC] = res.results[core]["outT"].T
    return out


if __name__ == "__main__":
    rng = np.random.default_rng(0)
    ins = {
        "entity_embedding": rng.standard_normal((500000, 4, FEAT), dtype=np.float32),
        "w1": rng.standard_normal((NHID, FEAT), dtype=np.float32) / np.sqrt(FEAT),
        "b1": rng.standard_normal((NHID,), dtype=np.float32) / np.sqrt(FEAT),
        "w2": rng.standard_normal((NOUT, NHID), dtype=np.float32) / np.sqrt(NHID),
        "b2": rng.standard_normal((NOUT,), dtype=np.float32) / np.sqrt(NHID),
        "idx0": rng.integers(0, 500000, B).astype(np.int32),
        "idx1": rng.integers(0, 4, B).astype(np.int32),
    }
    out = kernel(**ins)
    x = ins["entity_embedding"].reshape(NROWS, FEAT)[
        ins["idx0"].astype(np.int64) * 4 + ins["idx1"]]
    h = np.maximum(x @ ins["w1"].T + ins["b1"], 0.0)
    ref = np.maximum(h @ ins["w2"].T + ins["b2"], 0.0)
    err = np.abs(out - ref).max() / max(np.abs(ref).max(), 1e-9)
    print("rel err:", err)



# revision 3
# speedup vs baseline: 1.0090x; 1.0090x over previous
"""Trainium2 Bass kernel v3: embedding gather + 2-layer MLP, data-parallel x8.

Same gather structure as the 215us baseline (128 serial INDIRECT1D calls/core,
one 128-row gather each — the HW consumes one index per partition and calls
serialize on the GpSimd engine at ~1.1us, so the call count is fixed), but:

  - fp16 table: gather moves 256B/row instead of 512B.
  - All 128 gathers land in one full-size SBUF buffer (no pool recycling), so
    the gather stream free-runs with no buffer-wait gaps between calls.
  - fp16 MLP datapath (PE transposes fp16 in, fp32 PSUM accumulate).
  - Output accumulated in SBUF as fp16 [86, 16384] and written in 8 slabs
    alternating sync/scalar HWDGE rings and gpsimd SWDGE queues, instead of
    32 f32 writes all landing on the 2 SDMA engines of the sync ring.

Host: flat index preprocessing and the same slot map as the baseline;
output upcast fp16->f32 + transpose per core.
"""

import numpy as np
from contextlib import ExitStack

import concourse.bass as bass
import concourse.bacc as bacc
import concourse.tile as tile
from concourse import mybir
from concourse.bass_utils import run_bass_kernel_spmd
from concourse.masks import make_identity

F32 = mybir.dt.float32
F16 = mybir.dt.float16
I32 = mybir.dt.int32

N_CORES = 8
B = 131072
BC = B // N_CORES          # 16384 batch rows per core
FEAT = 128
NHID = 256
NOUT = 86
NROWS = 500000 * 4
P = 128
NJ = BC // P               # 128 j-columns of gathered rows per core
CHUNK_J = 4                # j-columns per MLP chunk (512 batch)
NCHUNK = NJ // CHUNK_J     # 32 chunks
SLAB = 4                   # chunks per output slab write (2048 cols)


def _build_program():
    nc = bacc.Bacc("TRN2", num_devices=N_CORES, num_swdge_queues=4)

    table = nc.dram_tensor("table", [NROWS, FEAT], F16, kind="ExternalInput").ap()
    idxs = nc.dram_tensor("idxs", [P, NJ], I32, kind="ExternalInput").ap()
    w1t = nc.dram_tensor("w1t", [FEAT, NHID], F16, kind="ExternalInput").ap()
    w2t = nc.dram_tensor("w2t", [P, NHID // P, NOUT], F16, kind="ExternalInput").ap()
    b1v = nc.dram_tensor("b1v", [P, NHID // P], F32, kind="ExternalInput").ap()
    b2v = nc.dram_tensor("b2v", [NOUT, 1], F32, kind="ExternalInput").ap()
    outT = nc.dram_tensor("outT", [NOUT, BC], F16, kind="ExternalOutput").ap()

    with tile.TileContext(nc) as tc, ExitStack() as ctx:
        const = ctx.enter_context(tc.tile_pool(name="const", bufs=1))
        xpool = ctx.enter_context(tc.tile_pool(name="xt", bufs=3))
        hpool = ctx.enter_context(tc.tile_pool(name="ht", bufs=3))
        psum = ctx.enter_context(tc.tile_pool(name="psum", bufs=2, space="PSUM"))

        idt = const.tile([P, P], F16)
        make_identity(nc, idt[:])

        w1t_t = const.tile([FEAT, NHID], F16)
        nc.sync.dma_start(w1t_t[:], w1t[:])
        w2t_t = const.tile([P, NHID // P, NOUT], F16)
        nc.sync.dma_start(w2t_t[:], w2t[:])
        b1_t = const.tile([P, NHID // P], F32)
        nc.sync.dma_start(b1_t[:], b1v[:])
        b2_t = const.tile([NOUT, 1], F32)
        nc.sync.dma_start(b2_t[:], b2v[:])
        idx_t = const.tile([P, NJ], I32)
        nc.scalar.dma_start(idx_t[:], idxs[:])

        # one resident gather buffer: 128 j-cols x 128 feat fp16 = 32KB/part
        xbuf = const.tile([P, NJ, FEAT], F16)
        # resident output accumulator [86, 16384] fp16 = 32KB/part
        obuf = const.tile([NOUT, BC], F16)

        for j in range(NJ):
            g = nc.gpsimd.indirect_dma_start(
                out=xbuf[:, j, :],
                out_offset=None,
                in_=table[:],
                in_offset=bass.IndirectOffsetOnAxis(ap=idx_t[:, j:j + 1], axis=0),
            )
            g.ins.queue = f"qPoolDynamic{j % 4 or ''}"

        for c in range(NCHUNK):
            bcol = c * CHUNK_J * P
            xtr = psum.tile([P, CHUNK_J * P], F16, tag="xtr")
            for i in range(CHUNK_J):
                nc.tensor.transpose(
                    out=xtr[:, i * P:(i + 1) * P],
                    in_=xbuf[:, c * CHUNK_J + i, :],
                    identity=idt[:],
                )
            xt = xpool.tile([P, CHUNK_J * P], F16)
            nc.vector.tensor_copy(out=xt[:], in_=xtr[:])

            hp = psum.tile([P, NHID // P, CHUNK_J * P], F32, tag="h")
            for k in range(NHID // P):
                nc.tensor.matmul(
                    out=hp[:, k, :],
                    lhsT=w1t_t[:, k * P:(k + 1) * P],
                    rhs=xt[:],
                    start=True, stop=True,
                )
            ht = hpool.tile([P, NHID // P, CHUNK_J * P], F16)
            nc.scalar.activation(
                out=ht[:, 0, :], in_=hp[:, 0, :],
                func=mybir.ActivationFunctionType.Relu,
                bias=b1_t[:, 0:1],
            )
            nc.vector.tensor_scalar(
                out=ht[:, 1, :], in0=hp[:, 1, :],
                scalar1=b1_t[:, 1:2], scalar2=0.0,
                op0=mybir.AluOpType.add, op1=mybir.AluOpType.max,
            )

            op_ = psum.tile([NOUT, CHUNK_J * P], F32, tag="ot")
            for k in range(NHID // P):
                nc.tensor.matmul(
                    out=op_[:],
                    lhsT=w2t_t[:, k, :],
                    rhs=ht[:, k, :],
                    start=(k == 0), stop=(k == NHID // P - 1),
                )
            nc.scalar.activation(
                out=obuf[:, bcol:bcol + CHUNK_J * P], in_=op_[:],
                func=mybir.ActivationFunctionType.Relu,
                bias=b2_t[:],
            )

            if c % SLAB == SLAB - 1:
                lo = (c - SLAB + 1) * CHUNK_J * P
                hi = (c + 1) * CHUNK_J * P
                s = c // SLAB
                eng = [nc.sync, nc.scalar, nc.gpsimd, nc.gpsimd,
                       nc.sync, nc.scalar, nc.gpsimd, nc.gpsimd][s]
                d = eng.dma_start(outT[:, lo:hi], obuf[:, lo:hi])
                if eng is nc.gpsimd:
                    q = (2 + s) % 4
                    d.ins.queue = f"qPoolDynamic{q or ''}"

    nc.compile()
    return nc


TRACE = False
RUN_KWARGS = None
LAST = None

_SLOT_TO_BATCH = None


def _slot_map():
    global _SLOT_TO_BATCH
    if _SLOT_TO_BATCH is None:
        pp, jj = np.meshgrid(np.arange(P), np.arange(NJ), indexing="ij")
        cc = jj // CHUNK_J
        ii = jj % CHUNK_J
        _SLOT_TO_BATCH = cc * (CHUNK_J * P) + ii * P + pp  # [P, NJ]
    return _SLOT_TO_BATCH


def kernel(entity_embedding, w1, b1, w2, b2, idx0, idx1):
    table = np.asarray(entity_embedding, dtype=np.float32) \
        .reshape(NROWS, FEAT).astype(np.float16)
    flat_idx = (np.asarray(idx0, dtype=np.int64) * 4
                + np.asarray(idx1, dtype=np.int64)).astype(np.int32)
    w1tf = np.ascontiguousarray(np.asarray(w1, dtype=np.float32).T) \
        .astype(np.float16)
    w2tf = np.ascontiguousarray(
        np.asarray(w2, dtype=np.float32).T.reshape(NHID // P, P, NOUT)
        .transpose(1, 0, 2)).astype(np.float16)
    b1v = np.ascontiguousarray(
        np.asarray(b1, dtype=np.float32).reshape(NHID // P, P).T)
    b2v = np.ascontiguousarray(np.asarray(b2, dtype=np.float32).reshape(NOUT, 1))

    slot = _slot_map()
    in_maps = []
    for core in range(N_CORES):
        local = flat_idx[core * BC:(core + 1) * BC]
        idxs = np.ascontiguousarray(local[slot])  # [P, NJ] int32
        in_maps.append({
            "table": table,
            "idxs": idxs,
            "w1t": w1tf,
            "w2t": w2tf,
            "b1v": b1v,
            "b2v": b2v,
        })

    nc = _build_program()
    global LAST
    res = run_bass_kernel_spmd(
        nc, in_maps, core_ids=list(range(N_CORES)), trace=TRACE,
        **(RUN_KWARGS or {}),
    )
    LAST = res
    out = np.empty((B, NOUT), dtype=np.float32)
    for core in range(N_CORES):
        out[core * BC:(core + 1) * BC] = \
            np.asarray(res.results[core]["outT"], dtype=np.float32).T
    return out


# revision 6
# speedup vs baseline: 1.0509x; 1.0415x over previous
"""Trainium2 Bass kernel v3: embedding gather + 2-layer MLP, data-parallel x8.

Same gather structure as the 215us baseline (128 serial INDIRECT1D calls/core,
one 128-row gather each — the HW consumes one index per partition and calls
serialize on the GpSimd engine at ~1.1us, so the call count is fixed), but:

  - fp16 table: gather moves 256B/row instead of 512B.
  - All 128 gathers land in one full-size SBUF buffer (no pool recycling), so
    the gather stream free-runs with no buffer-wait gaps between calls.
  - fp16 MLP datapath (PE transposes fp16 in, fp32 PSUM accumulate).
  - Output accumulated in SBUF as fp16 [86, 16384] and written in 8 slabs
    alternating sync/scalar HWDGE rings and gpsimd SWDGE queues, instead of
    32 f32 writes all landing on the 2 SDMA engines of the sync ring.

Host: flat index preprocessing and the same slot map as the baseline;
output upcast fp16->f32 + transpose per core.
"""

import numpy as np
from contextlib import ExitStack

import concourse.bass as bass
import concourse.bacc as bacc
import concourse.tile as tile
from concourse import mybir
from concourse.bass_utils import run_bass_kernel_spmd
from concourse.masks import make_identity

F32 = mybir.dt.float32
F16 = mybir.dt.float16
I32 = mybir.dt.int32

N_CORES = 8
B = 131072
BC = B // N_CORES          # 16384 batch rows per core
FEAT = 128
NHID = 256
NOUT = 86
NROWS = 500000 * 4
P = 128
NJ = BC // P               # 128 j-columns of gathered rows per core
CHUNK_J = 4                # j-columns per MLP chunk (512 batch)
NCHUNK = NJ // CHUNK_J     # 32 chunks
SLAB = 2                   # chunks per output slab write (1024 cols)


def _build_program():
    nc = bacc.Bacc("TRN2", num_devices=N_CORES, num_swdge_queues=4)

    table = nc.dram_tensor("table", [NROWS, FEAT], F16, kind="ExternalInput").ap()
    idxs = nc.dram_tensor("idxs", [P, NJ], I32, kind="ExternalInput").ap()
    w1t = nc.dram_tensor("w1t", [FEAT, NHID], F16, kind="ExternalInput").ap()
    w2t = nc.dram_tensor("w2t", [P, NHID // P, NOUT], F16, kind="ExternalInput").ap()
    b1v = nc.dram_tensor("b1v", [P, NHID // P], F32, kind="ExternalInput").ap()
    b2v = nc.dram_tensor("b2v", [NOUT, 1], F32, kind="ExternalInput").ap()
    outT = nc.dram_tensor("outT", [NOUT, BC], F16, kind="ExternalOutput").ap()

    with tile.TileContext(nc) as tc, ExitStack() as ctx:
        const = ctx.enter_context(tc.tile_pool(name="const", bufs=1))
        xpool = ctx.enter_context(tc.tile_pool(name="xt", bufs=3))
        hpool = ctx.enter_context(tc.tile_pool(name="ht", bufs=3))
        psum = ctx.enter_context(tc.tile_pool(name="psum", bufs=2, space="PSUM"))

        idx_t = const.tile([P, NJ], I32)
        nc.sync.dma_start(idx_t[:], idxs[:])

        idt = const.tile([P, P], F16)
        make_identity(nc, idt[:])

        w1t_t = const.tile([FEAT, NHID], F16)
        nc.scalar.dma_start(w1t_t[:], w1t[:])
        w2t_t = const.tile([P, NHID // P, NOUT], F16)
        nc.scalar.dma_start(w2t_t[:], w2t[:])
        b1_t = const.tile([P, NHID // P], F32)
        nc.scalar.dma_start(b1_t[:], b1v[:])
        b2_t = const.tile([NOUT, 1], F32)
        nc.scalar.dma_start(b2_t[:], b2v[:])

        # one resident gather buffer: 128 j-cols x 128 feat fp16 = 32KB/part
        xbuf = const.tile([P, NJ, FEAT], F16)
        # resident output accumulator [86, 16384] fp16 = 32KB/part
        obuf = const.tile([NOUT, BC], F16)

        for j in range(NJ):
            g = nc.gpsimd.indirect_dma_start(
                out=xbuf[:, j, :],
                out_offset=None,
                in_=table[:],
                in_offset=bass.IndirectOffsetOnAxis(ap=idx_t[:, j:j + 1], axis=0),
            )
            g.ins.queue = f"qPoolDynamic{j % 4 or ''}"

        for c in range(NCHUNK):
            bcol = c * CHUNK_J * P
            xtr = psum.tile([P, CHUNK_J * P], F16, tag="xtr")
            for i in range(CHUNK_J):
                nc.tensor.transpose(
                    out=xtr[:, i * P:(i + 1) * P],
                    in_=xbuf[:, c * CHUNK_J + i, :],
                    identity=idt[:],
                )
            xt = xpool.tile([P, CHUNK_J * P], F16)
            nc.vector.tensor_copy(out=xt[:], in_=xtr[:])

            hp = psum.tile([P, NHID // P, CHUNK_J * P], F32, tag="h")
            for k in range(NHID // P):
                nc.tensor.matmul(
                    out=hp[:, k, :],
                    lhsT=w1t_t[:, k * P:(k + 1) * P],
                    rhs=xt[:],
                    start=True, stop=True,
                )
            ht = hpool.tile([P, NHID // P, CHUNK_J * P], F16)
            nc.scalar.activation(
                out=ht[:, 0, :], in_=hp[:, 0, :],
                func=mybir.ActivationFunctionType.Relu,
                bias=b1_t[:, 0:1],
            )
            nc.vector.tensor_scalar(
                out=ht[:, 1, :], in0=hp[:, 1, :],
                scalar1=b1_t[:, 1:2], scalar2=0.0,
                op0=mybir.AluOpType.add, op1=mybir.AluOpType.max,
            )

            op_ = psum.tile([NOUT, CHUNK_J * P], F32, tag="ot")
            for k in range(NHID // P):
                nc.tensor.matmul(
                    out=op_[:],
                    lhsT=w2t_t[:, k, :],
                    rhs=ht[:, k, :],
                    start=(k == 0), stop=(k == NHID // P - 1),
                )
            nc.scalar.activation(
                out=obuf[:, bcol:bcol + CHUNK_J * P], in_=op_[:],
                func=mybir.ActivationFunctionType.Relu,
                bias=b2_t[:],
            )

            if c % SLAB == SLAB - 1:
                lo = (c - SLAB + 1) * CHUNK_J * P
                hi = (c + 1) * CHUNK_J * P
                s = c // SLAB
                eng = nc.sync if s % 2 == 0 else nc.scalar
                eng.dma_start(outT[:, lo:hi], obuf[:, lo:hi])

    nc.compile()
    return nc


TRACE = False
RUN_KWARGS = None
LAST = None

_SLOT_TO_BATCH = None


def _slot_map():
    global _SLOT_TO_BATCH
    if _SLOT_TO_BATCH is None:
        pp, jj = np.meshgrid(np.arange(P), np.arange(NJ), indexing="ij")
        cc = jj // CHUNK_J
        ii = jj % CHUNK_J
        _SLOT_TO_BATCH = cc * (CHUNK_J * P) + ii * P + pp  # [P, NJ]
    return _SLOT_TO_BATCH


def kernel(entity_embedding, w1, b1, w2, b2, idx0, idx1):
    table = np.asarray(entity_embedding, dtype=np.float32) \
        .reshape(NROWS, FEAT).astype(np.float16)
    flat_idx = (np.asarray(idx0, dtype=np.int64) * 4
                + np.asarray(idx1, dtype=np.int64)).astype(np.int32)
    w1tf = np.ascontiguousarray(np.asarray(w1, dtype=np.float32).T) \
        .astype(np.float16)
    w2tf = np.ascontiguousarray(
        np.asarray(w2, dtype=np.float32).T.reshape(NHID // P, P, NOUT)
        .transpose(1, 0, 2)).astype(np.float16)
    b1v = np.ascontiguousarray(
        np.asarray(b1, dtype=np.float32).reshape(NHID // P, P).T)
    b2v = np.ascontiguousarray(np.asarray(b2, dtype=np.float32).reshape(NOUT, 1))

    slot = _slot_map()
    in_maps = []
    for core in range(N_CORES):
        local = flat_idx[core * BC:(core + 1) * BC]
        idxs = np.ascontiguousarray(local[slot])  # [P, NJ] int32
        in_maps.append({
            "table": table,
            "idxs": idxs,
            "w1t": w1tf,
            "w2t": w2tf,
            "b1v": b1v,
            "b2v": b2v,
        })

    nc = _build_program()
    global LAST
    res = run_bass_kernel_spmd(
        nc, in_maps, core_ids=list(range(N_CORES)), trace=TRACE,
        **(RUN_KWARGS or {}),
    )
    LAST = res
    out = np.empty((B, NOUT), dtype=np.float32)
    for core in range(N_CORES):
        out[core * BC:(core + 1) * BC] = \
            np.asarray(res.results[core]["outT"], dtype=np.float32).T
    return out


# revision 11
# speedup vs baseline: 1.0534x; 1.0024x over previous
"""Trainium2 Bass kernel v3: embedding gather + 2-layer MLP, data-parallel x8.

Same gather structure as the 215us baseline (128 serial INDIRECT1D calls/core,
one 128-row gather each — the HW consumes one index per partition and calls
serialize on the GpSimd engine at ~1.1us, so the call count is fixed), but:

  - fp16 table: gather moves 256B/row instead of 512B.
  - All 128 gathers land in one full-size SBUF buffer (no pool recycling), so
    the gather stream free-runs with no buffer-wait gaps between calls.
  - fp16 MLP datapath (PE transposes fp16 in, fp32 PSUM accumulate).
  - Output accumulated in SBUF as fp16 [86, 16384] and written in 8 slabs
    alternating sync/scalar HWDGE rings and gpsimd SWDGE queues, instead of
    32 f32 writes all landing on the 2 SDMA engines of the sync ring.

Host: flat index preprocessing and the same slot map as the baseline;
output upcast fp16->f32 + transpose per core.
"""

import numpy as np
from contextlib import ExitStack

import concourse.bass as bass
import concourse.bacc as bacc
import concourse.tile as tile
from concourse import mybir
from concourse.bass_utils import run_bass_kernel_spmd

F32 = mybir.dt.float32
F16 = mybir.dt.float16
I32 = mybir.dt.int32

N_CORES = 8
B = 131072
BC = B // N_CORES          # 16384 batch rows per core
FEAT = 128
NHID = 256
NOUT = 86
NROWS = 500000 * 4
P = 128
NJ = BC // P               # 128 j-columns of gathered rows per core
CHUNK_J = 4                # j-columns per MLP chunk (512 batch)
NCHUNK = NJ // CHUNK_J     # 32 chunks
SLAB = 2                   # chunks per output slab write (1024 cols)


def _build_program():
    nc = bacc.Bacc("TRN2", num_devices=N_CORES, num_swdge_queues=4)

    table = nc.dram_tensor("table", [NROWS, FEAT], F16, kind="ExternalInput").ap()
    idxs = nc.dram_tensor("idxs", [P, NJ], I32, kind="ExternalInput").ap()
    w1t = nc.dram_tensor("w1t", [FEAT, NHID], F16, kind="ExternalInput").ap()
    w2t = nc.dram_tensor("w2t", [P, NHID // P, NOUT], F16, kind="ExternalInput").ap()
    b1v = nc.dram_tensor("b1v", [P, NHID // P], F32, kind="ExternalInput").ap()
    b2v = nc.dram_tensor("b2v", [NOUT, 1], F32, kind="ExternalInput").ap()
    idtv = nc.dram_tensor("idtv", [P, P], F16, kind="ExternalInput").ap()
    outT = nc.dram_tensor("outT", [NOUT, BC], F16, kind="ExternalOutput").ap()

    with tile.TileContext(nc) as tc, ExitStack() as ctx:
        const = ctx.enter_context(tc.tile_pool(name="const", bufs=1))
        xpool = ctx.enter_context(tc.tile_pool(name="xt", bufs=3))
        hpool = ctx.enter_context(tc.tile_pool(name="ht", bufs=3))
        psum = ctx.enter_context(tc.tile_pool(name="psum", bufs=2, space="PSUM"))

        idx_t = const.tile([P, NJ], I32)
        for k in range(4):
            ks = slice(k * (NJ // 4), (k + 1) * (NJ // 4))
            nc.sync.dma_start(idx_t[:, ks], idxs[:, ks])

        idt = const.tile([P, P], F16)
        nc.scalar.dma_start(idt[:], idtv[:])

        w1t_t = const.tile([FEAT, NHID], F16)
        nc.scalar.dma_start(w1t_t[:], w1t[:])
        w2t_t = const.tile([P, NHID // P, NOUT], F16)
        nc.scalar.dma_start(w2t_t[:], w2t[:])
        b1_t = const.tile([P, NHID // P], F32)
        nc.scalar.dma_start(b1_t[:], b1v[:])
        b2_t = const.tile([NOUT, 1], F32)
        nc.scalar.dma_start(b2_t[:], b2v[:])

        # one resident gather buffer: 128 j-cols x 128 feat fp16 = 32KB/part
        xbuf = const.tile([P, NJ, FEAT], F16)
        # resident output accumulator [86, 16384] fp16 = 32KB/part
        obuf = const.tile([NOUT, BC], F16)

        for j in range(NJ):
            g = nc.gpsimd.indirect_dma_start(
                out=xbuf[:, j, :],
                out_offset=None,
                in_=table[:],
                in_offset=bass.IndirectOffsetOnAxis(ap=idx_t[:, j:j + 1], axis=0),
            )
            g.ins.queue = f"qPoolDynamic{j % 4 or ''}"

        for c in range(NCHUNK):
            bcol = c * CHUNK_J * P
            xtr = psum.tile([P, CHUNK_J * P], F16, tag="xtr")
            for i in range(CHUNK_J):
                nc.tensor.transpose(
                    out=xtr[:, i * P:(i + 1) * P],
                    in_=xbuf[:, c * CHUNK_J + i, :],
                    identity=idt[:],
                )
            xt = xpool.tile([P, CHUNK_J * P], F16)
            nc.vector.tensor_copy(out=xt[:], in_=xtr[:])

            hp = psum.tile([P, NHID // P, CHUNK_J * P], F32, tag="h")
            for k in range(NHID // P):
                nc.tensor.matmul(
                    out=hp[:, k, :],
                    lhsT=w1t_t[:, k * P:(k + 1) * P],
                    rhs=xt[:],
                    start=True, stop=True,
                )
            ht = hpool.tile([P, NHID // P, CHUNK_J * P], F16)
            nc.scalar.activation(
                out=ht[:, 0, :], in_=hp[:, 0, :],
                func=mybir.ActivationFunctionType.Relu,
                bias=b1_t[:, 0:1],
            )
            nc.vector.tensor_scalar(
                out=ht[:, 1, :], in0=hp[:, 1, :],
                scalar1=b1_t[:, 1:2], scalar2=0.0,
                op0=mybir.AluOpType.add, op1=mybir.AluOpType.max,
            )

            op_ = psum.tile([NOUT, CHUNK_J * P], F32, tag="ot")
            for k in range(NHID // P):
                nc.tensor.matmul(
                    out=op_[:],
                    lhsT=w2t_t[:, k, :],
                    rhs=ht[:, k, :],
                    start=(k == 0), stop=(k == NHID // P - 1),
                )
            nc.scalar.activation(
                out=obuf[:, bcol:bcol + CHUNK_J * P], in_=op_[:],
                func=mybir.ActivationFunctionType.Relu,
                bias=b2_t[:],
            )

            if c >= NCHUNK - 2:
                # drain the last two chunks individually on parallel rings
                lo = c * CHUNK_J * P
                hi = (c + 1) * CHUNK_J * P
                eng = nc.sync if c % 2 == 0 else nc.scalar
                eng.dma_start(outT[:, lo:hi], obuf[:, lo:hi])
            elif c % SLAB == SLAB - 1:
                lo = (c - SLAB + 1) * CHUNK_J * P
                hi = (c + 1) * CHUNK_J * P
                s = c // SLAB
                eng = nc.sync if s % 2 == 0 else nc.scalar
                eng.dma_start(outT[:, lo:hi], obuf[:, lo:hi])

    nc.compile()
    return nc


TRACE = False
RUN_KWARGS = None
LAST = None

_SLOT_TO_BATCH = None


def _slot_map():
    global _SLOT_TO_BATCH
    if _SLOT_TO_BATCH is None:
        pp, jj = np.meshgrid(np.arange(P), np.arange(NJ), indexing="ij")
        cc = jj // CHUNK_J
        ii = jj % CHUNK_J
        _SLOT_TO_BATCH = cc * (CHUNK_J * P) + ii * P + pp  # [P, NJ]
    return _SLOT_TO_BATCH


def kernel(entity_embedding, w1, b1, w2, b2, idx0, idx1):
    table = np.asarray(entity_embedding, dtype=np.float32) \
        .reshape(NROWS, FEAT).astype(np.float16)
    flat_idx = (np.asarray(idx0, dtype=np.int64) * 4
                + np.asarray(idx1, dtype=np.int64)).astype(np.int32)
    w1tf = np.ascontiguousarray(np.asarray(w1, dtype=np.float32).T) \
        .astype(np.float16)
    w2tf = np.ascontiguousarray(
        np.asarray(w2, dtype=np.float32).T.reshape(NHID // P, P, NOUT)
        .transpose(1, 0, 2)).astype(np.float16)
    b1v = np.ascontiguousarray(
        np.asarray(b1, dtype=np.float32).reshape(NHID // P, P).T)
    b2v = np.ascontiguousarray(np.asarray(b2, dtype=np.float32).reshape(NOUT, 1))

    slot = _slot_map()
    in_maps = []
    for core in range(N_CORES):
        local = flat_idx[core * BC:(core + 1) * BC]
        idxs = np.ascontiguousarray(local[slot])  # [P, NJ] int32
        in_maps.append({
            "table": table,
            "idxs": idxs,
            "w1t": w1tf,
            "w2t": w2tf,
            "b1v": b1v,
            "b2v": b2v,
            "idtv": np.eye(P, dtype=np.float16),
        })

    nc = _build_program()
    global LAST
    res = run_bass_kernel_spmd(
        nc, in_maps, core_ids=list(range(N_CORES)), trace=TRACE,
        **(RUN_KWARGS or {}),
    )
    LAST = res
    out = np.empty((B, NOUT), dtype=np.float32)
    for core in range(N_CORES):
        out[core * BC:(core + 1) * BC] = \
            np.asarray(res.results[core]["outT"], dtype=np.float32).T
    return out
